# revision 30
# baseline (speedup 1.0000x reference)
"""Multi-head attention on 8 TRN2 NeuronCores (tensor-parallel over heads).

Problem (hardcoded): x[4,2048,1024] f32, w_qkv[1024,3072], w_out[1024,1024],
b_out[1024]; 16 heads, dim_head 64; out = softmax(q k^T / 8) v @ w_out + b_out.

Strategy:
  - Core c owns heads (2c, 2c+1). Host passes x pre-transposed (xT[D,S], bf16)
    and per-core w_qkv column shards; everything on-chip stays in transposed
    [feature, seq] layout so no score-matrix transposes are needed:
      qkvT = wqkv_c^T @ x^T                  (per core [384, 8192])
      S^T  = k^T-tile^T-contraction: matmul(lhsT=kT, rhs=qT) -> [j, i]
      expS = exp(S^T * scale)  (no max-subtraction; inputs are N(0,1)-scaled)
      U^T  = matmul(lhsT=[v|1], rhs=expS) -> [65, i]; row 64 = softmax denom
      attnT = U^T[0:64] * bcast(1/denom)     (approx recip + ones-matmul bcast)
  - attn_groups software-pipelines the PE FIFO: av(g-1) is emitted after
    scores(g) and paced fillers so the engine never parks on an exp wait;
    boundary work (normalize + per-ic a2a staging + collective issue) is
    emitted as `pre` work early inside the NEXT ic's attention.
  - AllToAll redistributes attnT from head-sharded columns to row-sharded
    blocks (row unit u = 128 rows, owned by core u%8), one collective per
    batch, issued as soon as its last ic is staged so the wire time hides
    under the next batch; output projections are deferred fillers scheduled
    where their collective is guaranteed complete (proj(b0)@b2, proj(b1)@b3
    ic0-2, proj(b2)@b3 ic3 + tail, proj(b3) after the tail AllToAll).
  - Each core computes its row block of the output projection with the full
    w_out: outT[e, r] = w_out^T @ gathered + bias.
Host gathers per-core [1024,1024] outT blocks into the full [4,2048,1024].
"""

import numpy as np
import ml_dtypes

import concourse.bass as bass
import concourse.mybir as mybir
import concourse.tile as tile
from concourse import bacc
from concourse.bass_utils import run_bass_kernel_spmd
from concourse.masks import make_identity

BF16 = mybir.dt.bfloat16
F32 = mybir.dt.float32
AF = mybir.ActivationFunctionType

B, N, D, H, DH = 4, 2048, 1024, 16, 64
NCORES = 8
HL = H // NCORES          # heads per core (2)
SCALE = DH ** -0.5
S = B * N                 # 8192 global rows
RL = S // NCORES          # 1024 rows per core
KT = D // 128             # 8 contraction tiles
JT = N // 128             # 16 key tiles per batch
IC = 4                    # i-chunks per batch
ICW = N // IC             # 512
GW = 2                    # j-tiles per exp group
G = JT // GW              # 8 groups
M3 = 3 * HL * DH          # 384 qkv columns per core
VW = DH + 1               # v + ones column


def _build_kernel(nc, fake_collective=False, interleave=True,
                  pbcast=False, vacc_bufs=1, fast_recip=True):
    aux_bufs = 2 if vacc_bufs == 1 else 1  # PSUM budget: 4(sp)+2*vacc+aux = 8
    xT = nc.dram_tensor("xT", [D, S], BF16, kind="ExternalInput").ap()
    # weights host-prearranged to [128, t-major] so each partition's DMA read
    # is one contiguous run (6KB/16KB) instead of KT strided 768B/2KB pieces
    wqkv = nc.dram_tensor("wqkv", [128, KT * M3], BF16, kind="ExternalInput").ap()
    wout = nc.dram_tensor("wout", [128, KT * D], BF16, kind="ExternalInput").ap()
    bias = nc.dram_tensor("bias", [128, KT], F32, kind="ExternalInput").ap()
    out = nc.dram_tensor("out", [D, RL], F32, kind="ExternalOutput").ap()

    with (
        tile.TileContext(nc) as tc,
        tc.tile_pool(name="const", bufs=1) as constp,
        tc.tile_pool(name="xb", bufs=2) as xbp,
        tc.tile_pool(name="qkv", bufs=2) as qkvp,
        tc.tile_pool(name="vn", bufs=2) as vnp,
        tc.tile_pool(name="at", bufs=2) as atp,
        tc.tile_pool(name="ex", bufs=3) as expp,
        tc.tile_pool(name="sm", bufs=2) as smp,
        tc.tile_pool(name="ob", bufs=4) as obp,
        tc.tile_pool(name="psc", bufs=2, space="PSUM") as pscp,   # scores: 2 x [128,1024]
        tc.tile_pool(name="pva", bufs=vacc_bufs, space="PSUM") as pvap,  # vacc
        tc.tile_pool(name="pax", bufs=aux_bufs, space="PSUM") as paxp,  # aux
        tc.tile_pool(name="dram", bufs=2, space="DRAM") as dramp,
    ):
        wq_sb = constp.tile([128, KT, M3], BF16, name="wq_sb")
        nc.sync.dma_start(wq_sb[:], wqkv.rearrange("p (t m) -> p t m", t=KT))
        wo_sb = constp.tile([128, KT, D], BF16, name="wo_sb")

        def load_wout():
            # deferred: 2.1MB load not needed until the first projection, so
            # keep it off the startup critical path (x/wqkv loads)
            nc.sync.dma_start(wo_sb[:], wout.rearrange("p (t e) -> p t e", t=KT))
        bias_sb = constp.tile([128, KT], F32, name="bias_sb")
        nc.sync.dma_start(bias_sb[:], bias)
        ident = constp.tile([128, 128], BF16, name="ident")
        make_identity(nc, ident)
        ones64 = constp.tile([1, DH], BF16, name="ones64")
        nc.gpsimd.memset(ones64, 1.0)

        def load_xb(b, engine=None):
            eng = engine if engine is not None else nc.sync
            xb = xbp.tile([128, KT, N], BF16, name="xb", tag="xb")
            for sc in range(IC):
                eng.dma_start(
                    xb[:, :, sc * ICW:(sc + 1) * ICW],
                    xT.rearrange("(t p) s -> p t s", p=128)[
                        :, :, b * N + sc * ICW: b * N + (sc + 1) * ICW],
                )
            return xb

        def alloc_qkv():
            qt = qkvp.tile([128, N], BF16, name="qt", tag="qt")
            kt = qkvp.tile([128, N], BF16, name="kt", tag="kt")
            vt = qkvp.tile([128, N], BF16, name="vt", tag="vt")
            return qt, kt, vt

        def qkv_m(xb, dsts, sc, m):
            # one [128, 512] block of qkvT = wqkv^T @ xT
            ps = paxp.tile([128, ICW], F32, name="qkvps", tag="aux")
            for t in range(KT):
                nc.tensor.matmul(
                    ps,
                    wq_sb[:, t, m * 128:(m + 1) * 128],
                    xb[:, t, sc * ICW:(sc + 1) * ICW],
                    start=(t == 0), stop=(t == KT - 1),
                )
            nc.vector.tensor_copy(dsts[m][:, sc * ICW:(sc + 1) * ICW], ps)

        def alloc_vn():
            # per-j-tile natural v with a ones column per head:
            # lhsT for head h = vn[:, jt, h*VW:(h+1)*VW]
            vn = vnp.tile([128, JT, 2 * VW], BF16, name="vn", tag="vn")
            return vn

        def trans_jt(vn, vt, jt):
            # vT tile [128 dims(2 heads), 128 j] -> natural v [128 j, dims]
            # with a ones column appended per head (softmax denominator)
            ps = paxp.tile([128, 128], BF16, name="trps", tag="aux")
            nc.tensor.transpose(ps, vt[:, jt * 128:(jt + 1) * 128], ident)
            nc.vector.tensor_copy(vn[:, jt, 0:DH], ps[:, 0:DH])
            nc.vector.tensor_copy(vn[:, jt, VW:VW + DH], ps[:, DH:2 * DH])
            nc.vector.memset(vn[:, jt, DH:VW], 1.0)
            nc.vector.memset(vn[:, jt, VW + DH:2 * VW], 1.0)

        def attn_groups(qt, kt, vn, ic, fillers=(), pre=()):
            # fillers: callables emitting independent PE work, paced between
            # attention groups to backfill exp-gated stalls. pre: boundary
            # work (previous ic's normalize/staging/collective) emitted right
            # after group 0's scores -- before av(g0) is emitted, which is
            # required for the vacc WAR ordering with bufs=1
            fillers = list(fillers)
            emitted = 0
            vaccs = [
                pvap.tile([VW, ICW], F32, name=f"vacc{h}", tag=f"vacc{h}")
                for h in range(HL)
            ]
            qs = [qt[h * DH:(h + 1) * DH, ic * ICW:(ic + 1) * ICW] for h in range(HL)]

            def emit_av(exps, g):
                for w in range(GW):
                    jt = g * GW + w
                    for h in range(HL):
                        nc.tensor.matmul(
                            vaccs[h],
                            vn[:, jt, h * VW:(h + 1) * VW],
                            exps[h][:, w * ICW:(w + 1) * ICW],
                            start=(jt == 0), stop=(jt == JT - 1),
                        )

            pend = None  # software-pipeline: av(g-1) is emitted after
            # scores(g)+fillers, so the PE FIFO never parks on exp(g-1) —
            # the wait would block queued-behind filler MMs (strict FIFO)
            for g in range(G):
                sps = [
                    pscp.tile([128, GW * ICW], F32, name=f"sp{h}", tag="sp")
                    for h in range(HL)
                ]
                for w in range(GW):
                    jt = g * GW + w
                    for h in range(HL):
                        nc.tensor.matmul(
                            sps[h][:, w * ICW:(w + 1) * ICW],
                            kt[h * DH:(h + 1) * DH, jt * 128:(jt + 1) * 128],
                            qs[h], start=True, stop=True,
                        )
                exps = []
                for h in range(HL):
                    ex = expp.tile([128, GW * ICW], BF16, name=f"ex{h}", tag="ex")
                    nc.scalar.activation(ex, sps[h], AF.Exp, scale=SCALE)
                    exps.append(ex)
                if g == 0:
                    for f in pre:
                        f()
                want = (g + 1) * len(fillers) // G if interleave else 0
                while emitted < want:
                    fillers[emitted]()
                    emitted += 1
                if pend is not None:
                    emit_av(*pend)
                pend = (exps, g)
            emit_av(*pend)
            while emitted < len(fillers):
                fillers[emitted]()
                emitted += 1
            return vaccs

        def normalize_h(vaccs, at, ic, h):
            rc = smp.tile([1, ICW], F32, name="rc", tag="rc")
            # ~51 ULP approx is plenty for softmax denominators and ~5x
            # faster than the exact InstReciprocal (3.3us -> ~0.6us)
            if fast_recip:
                # custom-DVE op misbehaves reading PSUM directly; stage
                # the denominator row through SBUF first
                dn = smp.tile([1, ICW], F32, name="dn", tag="dn")
                nc.vector.tensor_copy(dn, vaccs[h][DH:VW, :])
                nc.vector.reciprocal_approx_fast(rc, dn)
            else:
                nc.vector.reciprocal(rc, vaccs[h][DH:VW, :])
            bc_sb = smp.tile([DH, ICW], F32, name="bc", tag="bc")
            if pbcast:
                # gpsimd broadcast shares the Pool queue with collectives;
                # an in-flight AllToAll head-of-line blocks it
                nc.gpsimd.partition_broadcast(bc_sb, rc)
            else:
                # ones-matmul broadcast in bf16 (1 col/cycle; plain fp32
                # operands would pay the 4-pass penalty). bf16 rounding of
                # 1/denom costs ~0.2% relative -- well inside tolerance.
                rc_bf = smp.tile([1, ICW], BF16, name="rc_bf", tag="rc_bf")
                nc.vector.tensor_copy(rc_bf, rc)
                bc_ps = paxp.tile([DH, ICW], F32, name="bcps", tag="aux")
                nc.tensor.matmul(bc_ps, ones64, rc_bf, start=True, stop=True)
                nc.vector.tensor_copy(bc_sb, bc_ps)
            nc.vector.tensor_mul(
                at[h * DH:(h + 1) * DH, ic * ICW:(ic + 1) * ICW],
                vaccs[h][0:DH, :], bc_sb,
            )

        def a2a(a_in, a_out):
            if fake_collective:
                nc.gpsimd.dma_start(a_out[:], a_in[:])
            else:
                nc.gpsimd.collective_compute(
                    "AllToAll", mybir.AluOpType.bypass,
                    replica_groups=[list(range(NCORES))],
                    ins=[a_in.opt()], outs=[a_out.opt()],
                )

        def stage_ic(dst_blocks, at, ic, half):
            # one ic's 512 finalized columns map to 4 dest cores: query
            # column i = hh*1024 + d*128 + x with hh = ic//2, d in the ic%2
            # half of dests. half=True targets [d][128,128] block buffers,
            # else the [d][128,256] full-batch buffers at column half hh.
            at4 = at.rearrange("p (hh d x) -> p hh d x", hh=2, d=NCORES)
            hh = ic // 2
            for dd in range((ic % 2) * 4, (ic % 2) * 4 + 4):
                dst = dst_blocks[dd] if half else \
                    dst_blocks[dd][:, hh * 128:(hh + 1) * 128]
                nc.sync.dma_start(dst, at4[:, hh, dd, :])

        def proj_load(a_out):
            # a_out [8 src, 128, 256] -> SBUF [128, src, 256]
            g_sb = obp.tile([128, NCORES, 256], BF16, name="g_sb", tag="g_sb", bufs=2)
            nc.sync.dma_start(g_sb[:], a_out.rearrange("s p r -> p s r"))
            return g_sb

        def proj_e(b, g_sb, e):
            ps = paxp.tile([128, 256], F32, name="prps", tag="aux")
            for s in range(NCORES):
                nc.tensor.matmul(
                    ps, wo_sb[:, s, e * 128:(e + 1) * 128], g_sb[:, s, :],
                    start=(s == 0), stop=(s == NCORES - 1),
                )
            ob = obp.tile([128, 256], F32, name="ob", tag="ob", bufs=4)
            nc.vector.tensor_scalar_add(ob, ps, bias_sb[:, e:e + 1])
            nc.sync.dma_start(out[e * 128:(e + 1) * 128, b * 256:(b + 1) * 256], ob)

        # ---- software-pipelined main flow ----
        # startup: xb(0) on the sync queue (vector queue would serialize
        # behind nothing here; sync is free) alongside the weight loads
        xb = load_xb(0, engine=nc.sync)
        cur = alloc_qkv()
        vn = alloc_vn()
        for sc in range(IC):
            for m in range(3):
                qkv_m(xb, cur, sc, m)
        for jt in range(JT):
            trans_jt(vn, cur[2], jt)
        load_wout()

        a_ins, a_outs, g_sbs = {}, {}, {}
        pend = []  # boundary work for the next attn_groups' pre slot
        for b in range(B):
            at = atp.tile([128, N], BF16, name="at", tag="at")
            a_ins[b] = dramp.tile([NCORES, 128, 256], BF16,
                                  name="a_in", tag="a_in", bufs=2)
            a_outs[b] = dramp.tile([NCORES, 128, 256], BF16,
                                   name="a_out", tag="a_out", bufs=2)
            if b + 1 < B:
                xb_n = load_xb(b + 1)
                nxt = alloc_qkv()
                vn_n = alloc_vn()
            if b == 2:
                g_sbs[0] = proj_load(a_outs[0])
            if b == 3:
                g_sbs[1] = proj_load(a_outs[1])
            for ic in range(IC):
                if b == 3 and ic == 3:
                    g_sbs[2] = proj_load(a_outs[2])
                fillers = []
                if b + 1 < B:
                    fillers += [
                        (lambda m=m: qkv_m(xb_n, nxt, ic, m)) for m in range(3)
                    ]
                    if ic >= 1:
                        fillers += [
                            (lambda jt=jt: trans_jt(vn_n, nxt[2], jt))
                            for jt in range(4 * (ic - 1), 4 * ic)
                        ]
                # deferred output projections, scheduled where their a2a is
                # guaranteed complete: proj(b0) during b2, proj(b1) during
                # b3 ic0-2, proj(b2) first half at b3 ic3 (rest at the tail)
                if b == 2:
                    fillers += [
                        (lambda e=e: proj_e(0, g_sbs[0], e))
                        for e in (2 * ic, 2 * ic + 1)
                    ]
                if b == 3:
                    sched = {0: range(0, 4), 1: range(4, 6), 2: range(6, 8)}
                    fillers += [
                        (lambda e=e: proj_e(1, g_sbs[1], e))
                        for e in sched.get(ic, ())
                    ]
                if b == 3 and ic == 3:
                    fillers += [
                        (lambda e=e: proj_e(2, g_sbs[2], e)) for e in range(4)
                    ]
                vaccs = attn_groups(cur[0], cur[1], vn, ic, fillers, pre=pend)
                # boundary work: normalize this ic + stage its 4 dest blocks,
                # then (at batch boundaries) issue the collective; all
                # emitted early inside the NEXT attn_groups so the PE FIFO
                # never parks on it and the collective gets maximum cover
                pend = [
                    (lambda h=h, v=vaccs, a=at, i=ic: normalize_h(v, a, i, h))
                    for h in range(HL)
                ]
                pend.append(lambda a=at, i=ic, bb=b:
                            stage_ic(a_ins[bb], a, i, half=False))
                if ic == IC - 1 and b < B - 1:
                    pend.append(lambda bb=b: a2a(a_ins[bb], a_outs[bb]))
            if b + 1 < B:
                # jts 0-11 were emitted as paced fillers inside ic1-ic3
                for jt in range(12, JT):
                    trans_jt(vn_n, nxt[2], jt)
                cur, vn = nxt, vn_n
                xb = xb_n
        # tail: emit the last ic's normalize+staging, launch the final
        # AllToAll, and backfill its flight with the deferred projections
        for f in pend:
            f()
        a2a(a_ins[B - 1], a_outs[B - 1])
        for e in range(4, KT):
            proj_e(2, g_sbs[2], e)
        g_sbs[3] = proj_load(a_outs[B - 1])
        for e in range(KT):
            proj_e(B - 1, g_sbs[3], e)

    nc.compile()
    return nc


_CACHE = {}


def get_nc():
    if "nc" not in _CACHE:
        import os
        kw = {}
        for flag, name in (("BK_PBCAST", "pbcast"),
                           ("BK_FASTRECIP", "fast_recip")):
            if flag in os.environ:
                kw[name] = bool(int(os.environ[flag]))
        nc = bacc.Bacc("TRN2", target_bir_lowering=False, debug=False,
                       num_devices=NCORES)
        _CACHE["nc"] = _build_kernel(nc, **kw)
    return _CACHE["nc"]


def make_in_maps(x, w_qkv, w_out, b_out):
    bf = ml_dtypes.bfloat16
    xT = np.ascontiguousarray(
        np.asarray(x, dtype=np.float32).reshape(S, D).T).astype(bf)
    w_qkv = np.asarray(w_qkv, dtype=np.float32)
    # [1024, E] -> [128, KT*E]: partition-major so each partition's SBUF load
    # is one contiguous DMA run
    wout_bf = np.ascontiguousarray(
        np.asarray(w_out, dtype=np.float32)
        .reshape(KT, 128, D).transpose(1, 0, 2).reshape(128, KT * D)).astype(bf)
    bias = np.ascontiguousarray(
        np.asarray(b_out, dtype=np.float32).reshape(KT, 128).T)
    in_maps = []
    for c in range(NCORES):
        lo, hi = (HL * c) * DH, (HL * c + HL) * DH
        wq_c = np.concatenate(
            [w_qkv[:, lo:hi], w_qkv[:, D + lo:D + hi], w_qkv[:, 2 * D + lo:2 * D + hi]],
            axis=1)
        wq_c = (wq_c.reshape(KT, 128, M3).transpose(1, 0, 2)
                .reshape(128, KT * M3)).astype(bf)
        in_maps.append({
            "xT": xT, "wqkv": np.ascontiguousarray(wq_c),
            "wout": wout_bf, "bias": bias,
        })
    return in_maps


def gather(results):
    out = np.empty((S, D), dtype=np.float32)
    for u in range(S // 128):
        c, t = u % NCORES, u // NCORES
        out[u * 128:(u + 1) * 128] = results[c]["out"][:, t * 128:(t + 1) * 128].T
    return out.reshape(B, N, D)


def run(x, w_qkv, w_out, b_out, trace=False):
    nc = get_nc()
    in_maps = make_in_maps(x, w_qkv, w_out, b_out)
    res = run_bass_kernel_spmd(nc, in_maps, core_ids=list(range(NCORES)),
                               trace=trace)
    return gather(res.results), res


def kernel(x, w_qkv, w_out, b_out):
    out, _ = run(x, w_qkv, w_out, b_out, trace=False)
    return out


def _build_trivial():
    """Minimal NEFF used to calibrate the fixed per-execution dispatch
    overhead of the PJRT path (~450us), which neuron-profile's on-silicon
    exec_time would not include."""
    nc = bacc.Bacc("TRN2", target_bir_lowering=False, debug=False,
                   num_devices=NCORES)
    i_ap = nc.dram_tensor("i", [128, 128], F32, kind="ExternalInput").ap()
    o_ap = nc.dram_tensor("out", [128, 128], F32, kind="ExternalOutput").ap()
    with tile.TileContext(nc) as tc:
        with tc.tile_pool(name="p", bufs=1) as p:
            t = p.tile([128, 128], F32)
            nc.sync.dma_start(t, i_ap)
            nc.sync.dma_start(o_ap, t)
    nc.compile()
    return nc


def _bench_nc(nc, in_maps, k_small=8, k_big=256, reps=9):
    import time
    import jax
    from jax.sharding import Mesh, PartitionSpec, NamedSharding
    from jax.experimental.shard_map import shard_map
    from concourse import bass2jax

    bass2jax.install_neuronx_cc_hook()
    partition_name = nc.partition_id_tensor.name if nc.partition_id_tensor else None
    in_names, out_names, out_avals, zero_outs = [], [], [], []
    for alloc in nc.m.functions[0].allocations:
        if not isinstance(alloc, mybir.MemoryLocationSet):
            continue
        name = alloc.memorylocations[0].name
        if alloc.kind == "ExternalInput":
            if name != partition_name:
                in_names.append(name)
        elif alloc.kind == "ExternalOutput":
            shape = tuple(alloc.tensor_shape)
            dtype = mybir.dt.np(alloc.dtype)
            out_names.append(name)
            out_avals.append(jax.core.ShapedArray(shape, dtype))
            zero_outs.append(np.zeros(shape, dtype))
    n_params = len(in_names)
    all_in_names = list(in_names) + list(out_names)
    if partition_name is not None:
        all_in_names.append(partition_name)

    def _b(*args):
        operands = list(args)
        if partition_name is not None:
            operands.append(bass2jax.partition_id_tensor())
        outs = bass2jax._bass_exec_p.bind(
            *operands,
            out_avals=tuple(out_avals),
            in_names=tuple(all_in_names),
            out_names=tuple(out_names),
            lowering_input_output_aliases=(),
            sim_require_finite=True,
            sim_require_nnan=True,
            nc=nc,
        )
        return tuple(outs)

    devices = jax.devices()[:NCORES]
    mesh = Mesh(np.asarray(devices), ("core",))
    n_args = n_params + len(zero_outs)
    in_specs = (PartitionSpec("core"),) * n_args
    out_specs = (PartitionSpec("core"),) * len(out_names)
    sharding = NamedSharding(mesh, PartitionSpec("core"))

    concat_in = [
        np.concatenate([np.asarray(in_maps[c][nm]) for c in range(NCORES)], axis=0)
        for nm in in_names
    ] + [np.zeros((NCORES * z.shape[0], *z.shape[1:]), z.dtype) for z in zero_outs]
    dev_in = [jax.device_put(a, sharding) for a in concat_in]

    f = bass2jax.fast_dispatch_compile(
        lambda: jax.jit(shard_map(_b, mesh=mesh, in_specs=in_specs,
                                  out_specs=out_specs, check_rep=False),
                        keep_unused=True).lower(*dev_in).compile())
    jax.block_until_ready(f(*dev_in))  # warm
    jax.block_until_ready(f(*dev_in))

    def t_async(n):
        # async-dispatch n executions, block once at the end: device-side the
        # n NEFF executions queue back-to-back, so the difference between two
        # n values isolates per-execution device time.
        t0 = time.perf_counter()
        outs = [f(*dev_in) for _ in range(n)]
        jax.block_until_ready(outs)
        return time.perf_counter() - t0

    times = {k: [] for k in (k_small, k_big)}
    for _ in range(reps):
        for k in (k_small, k_big):
            times[k].append(t_async(k))
    # the RPC floor is bimodal across calls; median lands both k in the
    # dominant mode so the slope cancels it reliably
    med = {k: sorted(ts)[len(ts) // 2] for k, ts in times.items()}
    per_exec = (med[k_big] - med[k_small]) / (k_big - k_small)
    return per_exec * 1e9, {"med": med, "all": times}


def bench(x, w_qkv, w_out, b_out, k_small=8, k_big=256, reps=9):
    """Returns (calibrated_exec_ns, details): per-execution wall time of the
    kernel NEFF minus the trivial-NEFF dispatch floor."""
    nc = get_nc()
    in_maps = make_in_maps(x, w_qkv, w_out, b_out)
    raw_ns, detail = _bench_nc(nc, in_maps, k_small, k_big, reps)
    triv = _build_trivial()
    tmaps = [{"i": np.zeros((128, 128), np.float32)} for _ in range(NCORES)]
    triv_ns, tdetail = _bench_nc(triv, tmaps, k_small, k_big, reps)
    return raw_ns - triv_ns, {"raw_ns": raw_ns, "trivial_ns": triv_ns,
                              "kernel": detail, "trivial": tdetail}



# revision 46
# speedup vs baseline: 1.4128x; 1.4128x over previous
"""Multi-head attention on 8 TRN2 NeuronCores (tensor-parallel over heads).

Problem (hardcoded): x[4,2048,1024] f32, w_qkv[1024,3072], w_out[1024,1024],
b_out[1024]; 16 heads, dim_head 64; out = softmax(q k^T / 8) v @ w_out + b_out.

Strategy:
  - Core c owns heads (2c, 2c+1). Host passes x pre-transposed (xT[D,S], bf16)
    and per-core w_qkv column shards; everything on-chip stays in transposed
    [feature, seq] layout so no score-matrix transposes are needed:
      qkvT = wqkv_c^T @ x^T                  (per core [384, 8192])
      S^T  = k^T-tile^T-contraction: matmul(lhsT=kT, rhs=qT) -> [j, i]
      expS = exp(S^T * scale)  (no max-subtraction; inputs are N(0,1)-scaled)
      U^T  = matmul(lhsT=[v|1], rhs=expS) -> [65, i]; row 64 = softmax denom
      attnT = U^T[0:64] * bcast(1/denom)     (approx recip + ones-matmul bcast)
  - attn_groups software-pipelines the PE FIFO: av(g-1) is emitted after
    scores(g) and paced fillers so the engine never parks on an exp wait;
    boundary work (normalize + per-ic a2a staging + collective issue) is
    emitted as `pre` work early inside the NEXT ic's attention.
  - AllToAll redistributes attnT from head-sharded columns to row-sharded
    blocks (row unit u = 128 rows, owned by core u%8), one collective per
    batch, issued as soon as its last ic is staged so the wire time hides
    under the next batch; output projections are deferred fillers scheduled
    where their collective is guaranteed complete (proj(b0)@b2, proj(b1)@b3
    ic0-2, proj(b2)@b3 ic3 + tail, proj(b3) after the tail AllToAll).
  - Each core computes its row block of the output projection with the full
    w_out: outT[e, r] = w_out^T @ gathered + bias.
Host gathers per-core [1024,1024] outT blocks into the full [4,2048,1024].
"""

import numpy as np
import ml_dtypes

import concourse.bass as bass
import concourse.mybir as mybir
import concourse.tile as tile
from concourse import bacc
from concourse.bass_utils import run_bass_kernel_spmd
from concourse.masks import make_identity

BF16 = mybir.dt.bfloat16
F32 = mybir.dt.float32
AF = mybir.ActivationFunctionType

B, N, D, H, DH = 4, 2048, 1024, 16, 64
NCORES = 8
HL = H // NCORES          # heads per core (2)
SCALE = DH ** -0.5
S = B * N                 # 8192 global rows
RL = S // NCORES          # 1024 rows per core
KT = D // 128             # 8 contraction tiles
JT = N // 128             # 16 key tiles per batch
IC = 4                    # i-chunks per batch
ICW = N // IC             # 512
GW = 2                    # j-tiles per exp group
G = JT // GW              # 8 groups
M3 = 3 * HL * DH          # 384 qkv columns per core
VW = DH + 1               # v + ones column


def _build_kernel(nc, fake_collective=False, interleave=True,
                  pbcast=False, vacc_bufs=1, fast_recip=True):
    aux_bufs = 2 if vacc_bufs == 1 else 1  # PSUM budget: 4(sp)+2*vacc+aux = 8
    xT = nc.dram_tensor("xT", [D, S], BF16, kind="ExternalInput").ap()
    # weights host-prearranged to [128, t-major] so each partition's DMA read
    # is one contiguous run (6KB/16KB) instead of KT strided 768B/2KB pieces
    wqkv = nc.dram_tensor("wqkv", [128, KT * M3], BF16, kind="ExternalInput").ap()
    wout = nc.dram_tensor("wout", [128, KT * D], BF16, kind="ExternalInput").ap()
    bias = nc.dram_tensor("bias", [128, KT], F32, kind="ExternalInput").ap()
    out = nc.dram_tensor("out", [D, RL], F32, kind="ExternalOutput").ap()

    with (
        tile.TileContext(nc) as tc,
        tc.tile_pool(name="const", bufs=1) as constp,
        tc.tile_pool(name="xb", bufs=2) as xbp,
        tc.tile_pool(name="qkv", bufs=2) as qkvp,
        tc.tile_pool(name="vn", bufs=2) as vnp,
        tc.tile_pool(name="at", bufs=2) as atp,
        tc.tile_pool(name="ex", bufs=3) as expp,
        tc.tile_pool(name="sm", bufs=2) as smp,
        tc.tile_pool(name="ob", bufs=4) as obp,
        tc.tile_pool(name="psc", bufs=2, space="PSUM") as pscp,   # scores: 2 x [128,1024]
        tc.tile_pool(name="pva", bufs=vacc_bufs, space="PSUM") as pvap,  # vacc
        tc.tile_pool(name="pax", bufs=aux_bufs, space="PSUM") as paxp,  # aux
        tc.tile_pool(name="dram", bufs=2, space="DRAM") as dramp,
    ):
        # weights on the Scalar trigger queue so their transfers run in
        # parallel with the xb chunk loads on the Sync queue -- the first
        # qkv matmul is gated by max(wq, xb-chunk0) instead of their sum
        wq_sb = constp.tile([128, KT, M3], BF16, name="wq_sb")
        nc.scalar.dma_start(wq_sb[:], wqkv.rearrange("p (t m) -> p t m", t=KT))
        wo_sb = constp.tile([128, KT, D], BF16, name="wo_sb")

        def load_wout():
            # deferred: 2.1MB load not needed until the first projection, so
            # keep it off the startup critical path (x/wqkv loads)
            nc.scalar.dma_start(wo_sb[:], wout.rearrange("p (t e) -> p t e", t=KT))
        bias_sb = constp.tile([128, KT], F32, name="bias_sb")
        nc.scalar.dma_start(bias_sb[:], bias)
        ident = constp.tile([128, 128], BF16, name="ident")
        make_identity(nc, ident)
        ones64 = constp.tile([1, DH], BF16, name="ones64")
        nc.gpsimd.memset(ones64, 1.0)

        def load_xb(b, engine=None):
            eng = engine if engine is not None else nc.sync
            xb = xbp.tile([128, KT, N], BF16, name="xb", tag="xb")
            for sc in range(IC):
                eng.dma_start(
                    xb[:, :, sc * ICW:(sc + 1) * ICW],
                    xT.rearrange("(t p) s -> p t s", p=128)[
                        :, :, b * N + sc * ICW: b * N + (sc + 1) * ICW],
                )
            return xb

        def alloc_qkv():
            qt = qkvp.tile([128, N], BF16, name="qt", tag="qt")
            kt = qkvp.tile([128, N], BF16, name="kt", tag="kt")
            vt = qkvp.tile([128, N], BF16, name="vt", tag="vt")
            return qt, kt, vt

        def qkv_m(xb, dsts, sc, m):
            # one [128, 512] block of qkvT = wqkv^T @ xT
            ps = paxp.tile([128, ICW], F32, name="qkvps", tag="aux")
            for t in range(KT):
                nc.tensor.matmul(
                    ps,
                    wq_sb[:, t, m * 128:(m + 1) * 128],
                    xb[:, t, sc * ICW:(sc + 1) * ICW],
                    start=(t == 0), stop=(t == KT - 1),
                )
            nc.vector.tensor_copy(dsts[m][:, sc * ICW:(sc + 1) * ICW], ps)

        def alloc_vn():
            # per-j-tile natural v with a ones column per head:
            # lhsT for head h = vn[:, jt, h*VW:(h+1)*VW]
            vn = vnp.tile([128, JT, 2 * VW], BF16, name="vn", tag="vn")
            return vn

        def trans_jt(vn, vt, jt):
            # vT tile [128 dims(2 heads), 128 j] -> natural v [128 j, dims]
            # with a ones column appended per head (softmax denominator)
            ps = paxp.tile([128, 128], BF16, name="trps", tag="aux")
            nc.tensor.transpose(ps, vt[:, jt * 128:(jt + 1) * 128], ident)
            nc.vector.tensor_copy(vn[:, jt, 0:DH], ps[:, 0:DH])
            nc.vector.tensor_copy(vn[:, jt, VW:VW + DH], ps[:, DH:2 * DH])
            nc.vector.memset(vn[:, jt, DH:VW], 1.0)
            nc.vector.memset(vn[:, jt, VW + DH:2 * VW], 1.0)

        def attn_groups(qt, kt, vn, ic, fillers=(), pre=()):
            # fillers: callables emitting independent PE work, paced between
            # attention groups to backfill exp-gated stalls. pre: boundary
            # work (previous ic's normalize/staging/collective) emitted right
            # after group 0's scores -- before av(g0) is emitted, which is
            # required for the vacc WAR ordering with bufs=1
            fillers = list(fillers)
            emitted = 0
            vaccs = [
                pvap.tile([VW, ICW], F32, name=f"vacc{h}", tag=f"vacc{h}")
                for h in range(HL)
            ]
            qs = [qt[h * DH:(h + 1) * DH, ic * ICW:(ic + 1) * ICW] for h in range(HL)]

            def emit_av(exps, g):
                for w in range(GW):
                    jt = g * GW + w
                    for h in range(HL):
                        nc.tensor.matmul(
                            vaccs[h],
                            vn[:, jt, h * VW:(h + 1) * VW],
                            exps[h][:, w * ICW:(w + 1) * ICW],
                            start=(jt == 0), stop=(jt == JT - 1),
                        )

            pend = None  # software-pipeline: av(g-1) is emitted after
            # scores(g)+fillers, so the PE FIFO never parks on exp(g-1) —
            # the wait would block queued-behind filler MMs (strict FIFO)
            for g in range(G):
                sps = [
                    pscp.tile([128, GW * ICW], F32, name=f"sp{h}", tag="sp")
                    for h in range(HL)
                ]
                for w in range(GW):
                    jt = g * GW + w
                    for h in range(HL):
                        nc.tensor.matmul(
                            sps[h][:, w * ICW:(w + 1) * ICW],
                            kt[h * DH:(h + 1) * DH, jt * 128:(jt + 1) * 128],
                            qs[h], start=True, stop=True,
                        )
                exps = []
                for h in range(HL):
                    ex = expp.tile([128, GW * ICW], BF16, name=f"ex{h}", tag="ex")
                    nc.scalar.activation(ex, sps[h], AF.Exp, scale=SCALE)
                    exps.append(ex)
                if g == 0:
                    for f in pre:
                        f()
                want = (g + 1) * len(fillers) // G if interleave else 0
                while emitted < want:
                    fillers[emitted]()
                    emitted += 1
                if pend is not None:
                    emit_av(*pend)
                pend = (exps, g)
            emit_av(*pend)
            while emitted < len(fillers):
                fillers[emitted]()
                emitted += 1
            return vaccs

        def normalize_h(vaccs, at, ic, h):
            # ~51 ULP approx is plenty for softmax denominators and ~5x
            # faster than the exact InstReciprocal. (The custom-DVE op
            # misbehaves reading PSUM directly; stage through SBUF first.)
            rc = smp.tile([1, ICW], F32, name="rc", tag="rc")
            if fast_recip:
                dn = smp.tile([1, ICW], F32, name="dn", tag="dn")
                nc.vector.tensor_copy(dn, vaccs[h][DH:VW, :])
                nc.vector.reciprocal_approx_fast(rc, dn)
            else:
                nc.vector.reciprocal(rc, vaccs[h][DH:VW, :])
            # bf16 ones-matmul broadcast (1 col/cycle; plain fp32 operands
            # would pay the 4-pass penalty); ~0.2% relative rounding is fine
            rc_bf = smp.tile([1, ICW], BF16, name="rc_bf", tag="rc_bf")
            nc.vector.tensor_copy(rc_bf, rc)
            bc_sb = smp.tile([DH, ICW], F32, name="bc", tag="bc")
            bc_ps = paxp.tile([DH, ICW], F32, name="bcps", tag="aux")
            nc.tensor.matmul(bc_ps, ones64, rc_bf, start=True, stop=True)
            nc.vector.tensor_copy(bc_sb, bc_ps)
            nc.vector.tensor_mul(
                at[h * DH:(h + 1) * DH, ic * ICW:(ic + 1) * ICW],
                vaccs[h][0:DH, :], bc_sb,
            )

        def a2a(a_in, a_out):
            if fake_collective:
                nc.gpsimd.dma_start(a_out[:], a_in[:])
            else:
                nc.gpsimd.collective_compute(
                    "AllToAll", mybir.AluOpType.bypass,
                    replica_groups=[list(range(NCORES))],
                    ins=[a_in.opt()], outs=[a_out.opt()],
                )

        def stage_ic(dst_blocks, at, ic, half):
            # one ic's 512 finalized columns map to 4 dest cores: query
            # column i = hh*1024 + d*128 + x with hh = ic//2, d in the ic%2
            # half of dests. half=True targets [d][128,128] block buffers,
            # else the [d][128,256] full-batch buffers at column half hh.
            at4 = at.rearrange("p (hh d x) -> p hh d x", hh=2, d=NCORES)
            hh = ic // 2
            for dd in range((ic % 2) * 4, (ic % 2) * 4 + 4):
                dst = dst_blocks[dd] if half else \
                    dst_blocks[dd][:, hh * 128:(hh + 1) * 128]
                nc.sync.dma_start(dst, at4[:, hh, dd, :])

        def proj_load(a_out):
            # a_out [8 src, 128, 256] -> SBUF [128, src, 256]
            g_sb = obp.tile([128, NCORES, 256], BF16, name="g_sb", tag="g_sb", bufs=2)
            nc.sync.dma_start(g_sb[:], a_out.rearrange("s p r -> p s r"))
            return g_sb

        def proj_e(b, g_sb, e):
            ps = paxp.tile([128, 256], F32, name="prps", tag="aux")
            for s in range(NCORES):
                nc.tensor.matmul(
                    ps, wo_sb[:, s, e * 128:(e + 1) * 128], g_sb[:, s, :],
                    start=(s == 0), stop=(s == NCORES - 1),
                )
            ob = obp.tile([128, 256], F32, name="ob", tag="ob", bufs=4)
            nc.vector.tensor_scalar_add(ob, ps, bias_sb[:, e:e + 1])
            nc.sync.dma_start(out[e * 128:(e + 1) * 128, b * 256:(b + 1) * 256], ob)

        # ---- software-pipelined main flow ----
        # startup: xb(0) on the sync queue (vector queue would serialize
        # behind nothing here; sync is free) alongside the weight loads
        xb = load_xb(0, engine=nc.sync)
        cur = alloc_qkv()
        vn = alloc_vn()
        for sc in range(IC):
            for m in range(3):
                qkv_m(xb, cur, sc, m)
        for jt in range(JT):
            trans_jt(vn, cur[2], jt)
        load_wout()

        a_ins, a_outs, g_sbs = {}, {}, {}
        pend = []  # boundary work for the next attn_groups' pre slot
        for b in range(B):
            at = atp.tile([128, N], BF16, name="at", tag="at")
            a_ins[b] = dramp.tile([NCORES, 128, 256], BF16,
                                  name="a_in", tag="a_in", bufs=2)
            a_outs[b] = dramp.tile([NCORES, 128, 256], BF16,
                                   name="a_out", tag="a_out", bufs=2)
            if b + 1 < B:
                xb_n = load_xb(b + 1)
                nxt = alloc_qkv()
                vn_n = alloc_vn()
            if b == 2:
                g_sbs[0] = proj_load(a_outs[0])
            if b == 3:
                g_sbs[1] = proj_load(a_outs[1])
            for ic in range(IC):
                if b == 3 and ic == 3:
                    g_sbs[2] = proj_load(a_outs[2])
                fillers = []
                if b + 1 < B:
                    fillers += [
                        (lambda m=m: qkv_m(xb_n, nxt, ic, m)) for m in range(3)
                    ]
                    # transposes lag one ic so the transpose never waits on
                    # the same ic's qkv m=2 evacuation copy
                    if ic >= 1:
                        fillers += [
                            (lambda jt=jt: trans_jt(vn_n, nxt[2], jt))
                            for jt in range(4 * (ic - 1), 4 * ic)
                        ]
                # deferred output projections, scheduled where their a2a is
                # guaranteed complete: proj(b0) during b2, proj(b1) during
                # b3 ic0-2, proj(b2) first half at b3 ic3 (rest at the tail)
                if b == 2:
                    fillers += [
                        (lambda e=e: proj_e(0, g_sbs[0], e))
                        for e in (2 * ic, 2 * ic + 1)
                    ]
                if b == 3:
                    sched = {0: range(0, 4), 1: range(4, 6), 2: range(6, 8)}
                    fillers += [
                        (lambda e=e: proj_e(1, g_sbs[1], e))
                        for e in sched.get(ic, ())
                    ]
                if b == 3 and ic == 3:
                    fillers += [
                        (lambda e=e: proj_e(2, g_sbs[2], e)) for e in range(4)
                    ]
                vaccs = attn_groups(cur[0], cur[1], vn, ic, fillers, pre=pend)
                # boundary work: normalize this ic + stage its 4 dest blocks,
                # then (at batch boundaries) issue the collective; all
                # emitted early inside the NEXT attn_groups so the PE FIFO
                # never parks on it and the collective gets maximum cover
                pend = [
                    (lambda h=h, v=vaccs, a=at, i=ic: normalize_h(v, a, i, h))
                    for h in range(HL)
                ]
                pend.append(lambda a=at, i=ic, bb=b:
                            stage_ic(a_ins[bb], a, i, half=False))
                if ic == IC - 1 and b < B - 1:
                    pend.append(lambda bb=b: a2a(a_ins[bb], a_outs[bb]))
            if b + 1 < B:
                # jts 0-11 were emitted as paced fillers inside ic1-ic3
                for jt in range(12, JT):
                    trans_jt(vn_n, nxt[2], jt)
                cur, vn = nxt, vn_n
                xb = xb_n
        # tail: emit the last ic's normalize+staging, launch the final
        # AllToAll, and backfill its flight with the deferred projections
        for f in pend:
            f()
        a2a(a_ins[B - 1], a_outs[B - 1])
        for e in range(4, KT):
            proj_e(2, g_sbs[2], e)
        g_sbs[3] = proj_load(a_outs[B - 1])
        for e in range(KT):
            proj_e(B - 1, g_sbs[3], e)

    nc.compile()
    return nc


_CACHE = {}


def get_nc():
    if "nc" not in _CACHE:
        import os
        kw = {}
        for flag, name in (("BK_PBCAST", "pbcast"),
                           ("BK_FASTRECIP", "fast_recip")):
            if flag in os.environ:
                kw[name] = bool(int(os.environ[flag]))
        nc = bacc.Bacc("TRN2", target_bir_lowering=False, debug=False,
                       num_devices=NCORES)
        _CACHE["nc"] = _build_kernel(nc, **kw)
    return _CACHE["nc"]


def make_in_maps(x, w_qkv, w_out, b_out):
    bf = ml_dtypes.bfloat16
    xT = np.ascontiguousarray(
        np.asarray(x, dtype=np.float32).reshape(S, D).T).astype(bf)
    w_qkv = np.asarray(w_qkv, dtype=np.float32)
    # [1024, E] -> [128, KT*E]: partition-major so each partition's SBUF load
    # is one contiguous DMA run
    wout_bf = np.ascontiguousarray(
        np.asarray(w_out, dtype=np.float32)
        .reshape(KT, 128, D).transpose(1, 0, 2).reshape(128, KT * D)).astype(bf)
    bias = np.ascontiguousarray(
        np.asarray(b_out, dtype=np.float32).reshape(KT, 128).T)
    in_maps = []
    for c in range(NCORES):
        lo, hi = (HL * c) * DH, (HL * c + HL) * DH
        wq_c = np.concatenate(
            [w_qkv[:, lo:hi], w_qkv[:, D + lo:D + hi], w_qkv[:, 2 * D + lo:2 * D + hi]],
            axis=1)
        wq_c = (wq_c.reshape(KT, 128, M3).transpose(1, 0, 2)
                .reshape(128, KT * M3)).astype(bf)
        in_maps.append({
            "xT": xT, "wqkv": np.ascontiguousarray(wq_c),
            "wout": wout_bf, "bias": bias,
        })
    return in_maps


def gather(results):
    out = np.empty((S, D), dtype=np.float32)
    for u in range(S // 128):
        c, t = u % NCORES, u // NCORES
        out[u * 128:(u + 1) * 128] = results[c]["out"][:, t * 128:(t + 1) * 128].T
    return out.reshape(B, N, D)


def run(x, w_qkv, w_out, b_out, trace=False):
    nc = get_nc()
    in_maps = make_in_maps(x, w_qkv, w_out, b_out)
    res = run_bass_kernel_spmd(nc, in_maps, core_ids=list(range(NCORES)),
                               trace=trace)
    return gather(res.results), res


def kernel(x, w_qkv, w_out, b_out):
    out, _ = run(x, w_qkv, w_out, b_out, trace=False)
    return out


def _build_trivial():
    """Minimal NEFF used to calibrate the fixed per-execution dispatch
    overhead of the PJRT path (~450us), which neuron-profile's on-silicon
    exec_time would not include."""
    nc = bacc.Bacc("TRN2", target_bir_lowering=False, debug=False,
                   num_devices=NCORES)
    i_ap = nc.dram_tensor("i", [128, 128], F32, kind="ExternalInput").ap()
    o_ap = nc.dram_tensor("out", [128, 128], F32, kind="ExternalOutput").ap()
    with tile.TileContext(nc) as tc:
        with tc.tile_pool(name="p", bufs=1) as p:
            t = p.tile([128, 128], F32)
            nc.sync.dma_start(t, i_ap)
            nc.sync.dma_start(o_ap, t)
    nc.compile()
    return nc


def _bench_nc(nc, in_maps, k_small=8, k_big=256, reps=9):
    import time
    import jax
    from jax.sharding import Mesh, PartitionSpec, NamedSharding
    from jax.experimental.shard_map import shard_map
    from concourse import bass2jax

    bass2jax.install_neuronx_cc_hook()
    partition_name = nc.partition_id_tensor.name if nc.partition_id_tensor else None
    in_names, out_names, out_avals, zero_outs = [], [], [], []
    for alloc in nc.m.functions[0].allocations:
        if not isinstance(alloc, mybir.MemoryLocationSet):
            continue
        name = alloc.memorylocations[0].name
        if alloc.kind == "ExternalInput":
            if name != partition_name:
                in_names.append(name)
        elif alloc.kind == "ExternalOutput":
            shape = tuple(alloc.tensor_shape)
            dtype = mybir.dt.np(alloc.dtype)
            out_names.append(name)
            out_avals.append(jax.core.ShapedArray(shape, dtype))
            zero_outs.append(np.zeros(shape, dtype))
    n_params = len(in_names)
    all_in_names = list(in_names) + list(out_names)
    if partition_name is not None:
        all_in_names.append(partition_name)

    def _b(*args):
        operands = list(args)
        if partition_name is not None:
            operands.append(bass2jax.partition_id_tensor())
        outs = bass2jax._bass_exec_p.bind(
            *operands,
            out_avals=tuple(out_avals),
            in_names=tuple(all_in_names),
            out_names=tuple(out_names),
            lowering_input_output_aliases=(),
            sim_require_finite=True,
            sim_require_nnan=True,
            nc=nc,
        )
        return tuple(outs)

    devices = jax.devices()[:NCORES]
    mesh = Mesh(np.asarray(devices), ("core",))
    n_args = n_params + len(zero_outs)
    in_specs = (PartitionSpec("core"),) * n_args
    out_specs = (PartitionSpec("core"),) * len(out_names)
    sharding = NamedSharding(mesh, PartitionSpec("core"))

    concat_in = [
        np.concatenate([np.asarray(in_maps[c][nm]) for c in range(NCORES)], axis=0)
        for nm in in_names
    ] + [np.zeros((NCORES * z.shape[0], *z.shape[1:]), z.dtype) for z in zero_outs]
    dev_in = [jax.device_put(a, sharding) for a in concat_in]

    f = bass2jax.fast_dispatch_compile(
        lambda: jax.jit(shard_map(_b, mesh=mesh, in_specs=in_specs,
                                  out_specs=out_specs, check_rep=False),
                        keep_unused=True).lower(*dev_in).compile())
    jax.block_until_ready(f(*dev_in))  # warm
    jax.block_until_ready(f(*dev_in))

    def t_async(n):
        # async-dispatch n executions, block once at the end: device-side the
        # n NEFF executions queue back-to-back, so the difference between two
        # n values isolates per-execution device time.
        t0 = time.perf_counter()
        outs = [f(*dev_in) for _ in range(n)]
        jax.block_until_ready(outs)
        return time.perf_counter() - t0

    times = {k: [] for k in (k_small, k_big)}
    for _ in range(reps):
        for k in (k_small, k_big):
            times[k].append(t_async(k))
    # the RPC floor is bimodal across calls; median lands both k in the
    # dominant mode so the slope cancels it reliably
    med = {k: sorted(ts)[len(ts) // 2] for k, ts in times.items()}
    per_exec = (med[k_big] - med[k_small]) / (k_big - k_small)
    return per_exec * 1e9, {"med": med, "all": times}


def bench(x, w_qkv, w_out, b_out, k_small=8, k_big=256, reps=9):
    """Returns (calibrated_exec_ns, details): per-execution wall time of the
    kernel NEFF minus the trivial-NEFF dispatch floor."""
    nc = get_nc()
    in_maps = make_in_maps(x, w_qkv, w_out, b_out)
    raw_ns, detail = _bench_nc(nc, in_maps, k_small, k_big, reps)
    triv = _build_trivial()
    tmaps = [{"i": np.zeros((128, 128), np.float32)} for _ in range(NCORES)]
    triv_ns, tdetail = _bench_nc(triv, tmaps, k_small, k_big, reps)
    return raw_ns - triv_ns, {"raw_ns": raw_ns, "trivial_ns": triv_ns,
                              "kernel": detail, "trivial": tdetail}

